# revision 22
# baseline (speedup 1.0000x reference)
"""Trainium2 Bass kernel for a 3-layer GCN (nn_GCNNet).

Strategy (8 NeuronCores, graph/data parallel):
- Destination nodes are sharded contiguously across the 8 cores (12500 each,
  padded to 12544 = 98 tiles of 128).
- Per layer: each core transforms its shard (H' = scale * (X @ W), scale folds
  the symmetric deg^-1/2 normalization), the shards are AllGather'd in 4
  node-quarters (pipelined), then each core aggregates its incident edges by
  gathering source rows with dma_gather (512B descriptors) and scatter-adding
  on the TensorEngine via one-hot matmuls accumulated in PSUM:
      psum[f, d] += sum_e gathered[e, f] * (dstloc[e] == d)
  The bias is injected as a K=1 matmul with rhs = sqrt(deg) so that the
  deg^-1/2 of the destination can be deferred (relu is positively homogeneous):
  x~ = relu(raw_agg + b*sqrtdeg); the deferred dinv is folded into the next
  layer's transform scale (dinv^2) and into the final logits scale (dinv).
- The classifier (concat -> linear -> log_softmax) is fused in: each layer's
  x~ tiles are matmul'd against the matching Wl block into an SBUF logits
  accumulator; the final phase applies dinv, bl and a batched log_softmax.

Everything data-dependent (edge counts per tile/chunk) is specialized into the
instruction stream at trace time; the per-(tile,chunk) group counts are the
max over the 8 cores so one SPMD program serves all cores (pad gathers fetch
row 0; pad one-hot columns use sentinel 255 so they contribute nothing).
"""

import os
import sys

import numpy as np

sys.path.insert(0, "/opt/trn_rl_repo")

P = 128
D = 128
L = 3
C = 10
NCORES = 8


def make_cfg(N=100000, E=1600000, shard=12500, qrows=3200, tb=6):
    nt = -(-shard // P)          # tiles per core
    cfg = dict(
        N=N, E=E,
        SHARD=shard,
        NT=nt,
        NTROWS=nt * P,
        QROWS=qrows,
        NQ=4,
        PADN=4 * qrows,
        CROWS=NCORES * qrows,
        TB=tb,
    )
    assert 4 * qrows >= nt * P
    assert NCORES * qrows <= 32767, "chunk rows must fit int16"
    assert N <= NCORES * shard
    return cfg


FULL_CFG = make_cfg()


# ---------------------------------------------------------------------------
# Host preprocessing
# ---------------------------------------------------------------------------

def preprocess(edge_index, cfg):
    N, SHARD, NT, NTROWS = cfg["N"], cfg["SHARD"], cfg["NT"], cfg["NTROWS"]
    QROWS, NQ, TB = cfg["QROWS"], cfg["NQ"], cfg["TB"]

    src = np.asarray(edge_index[0], dtype=np.int64)
    dst = np.asarray(edge_index[1], dtype=np.int64)
    self_idx = np.arange(N, dtype=np.int64)
    src = np.concatenate([src, self_idx])
    dst = np.concatenate([dst, self_idx])

    deg = np.bincount(dst, minlength=N).astype(np.float64)
    dinv = deg ** -0.5

    # src -> (chunk, row-within-chunk)
    r = src // SHARD
    loc = src - r * SHARD
    q = loc // QROWS
    pos = loc - q * QROWS
    erow = (r * QROWS + pos).astype(np.int32)
    echunk = q.astype(np.int32)

    core = (dst // SHARD).astype(np.int32)
    ld = (dst - core.astype(np.int64) * SHARD).astype(np.int32)
    tile = ld // P
    dloc = ld - tile * P

    # uniform group counts: max over cores of ceil(count / P)
    key = (core.astype(np.int64) * NT + tile) * NQ + echunk
    cnt = np.bincount(key, minlength=NCORES * NT * NQ).reshape(NCORES, NT, NQ)
    Gmax = -(-cnt // P)
    Gmax = Gmax.max(axis=0)            # [NT, NQ]
    S = Gmax * P                       # padded slots per (t, q)

    # padded stream layout: chunk-major, tile-minor
    S_qt = S.T                         # [NQ, NT]
    flat = S_qt.reshape(-1)
    offs = np.concatenate([[0], np.cumsum(flat)[:-1]]).reshape(NQ, NT)
    TOT = int(flat.sum())

    idx_streams = np.zeros((NCORES, 16, TOT // 16), np.int16)
    dl_streams = np.zeros((NCORES, P, TOT // P), np.int16)
    for c in range(NCORES):
        m = core == c
        t_c, q_c, e_c, d_c = tile[m], echunk[m], erow[m], dloc[m]
        order = np.lexsort((t_c, q_c))
        t_s, q_s, e_s, d_s = t_c[order], q_c[order], e_c[order], d_c[order]
        keys = q_s.astype(np.int64) * NT + t_s
        if len(keys):
            change = np.concatenate([[True], keys[1:] != keys[:-1]])
            run_id = np.cumsum(change) - 1
            run_starts = np.flatnonzero(change)
            rank = np.arange(len(keys)) - run_starts[run_id]
            dest = offs[q_s, t_s] + rank
        else:
            dest = np.zeros(0, np.int64)
        pidx = np.zeros(TOT, np.int16)
        pdl = np.full(TOT, 255, np.int16)
        pidx[dest] = e_s.astype(np.int16)
        pdl[dest] = d_s.astype(np.int16)
        idx_streams[c] = pidx.reshape(-1, 16).T
        dl_streams[c] = pdl.reshape(-1, P).T

    # batch metadata (uniform across cores)
    batches = []
    icol = gcol = 0
    for qq in range(NQ):
        for b0 in range(0, NT, TB):
            tl = [(t, int(Gmax[t, qq]))
                  for t in range(b0, min(b0 + TB, NT)) if Gmax[t, qq] > 0]
            ni = sum(g for _, g in tl) * P
            if ni == 0:
                continue
            batches.append(dict(q=qq, tiles=tl, ni=ni, icol=icol, gcol=gcol))
            icol += ni // 16
            gcol += ni // P
    assert icol == TOT // 16 and gcol == TOT // P

    nz = Gmax > 0
    first_q = np.where(nz.any(axis=1), nz.argmax(axis=1), -1)
    last_q = np.where(nz.any(axis=1), NQ - 1 - nz[:, ::-1].argmax(axis=1), -1)
    gb_max = max(b["ni"] // P for b in batches)

    # per-core scale vectors; sqrtdeg packed on partitions {0,32,64} (matmul
    # operands must start at base partition 0/32/64)
    NC3 = -(-NT // 3)
    sq_pack = np.zeros((NCORES, 65, NC3 * P), np.float32)
    scale_cols = np.zeros((NCORES, P, L * NT), np.float32)
    dinv_cols = np.zeros((NCORES, P, NT), np.float32)
    for c in range(NCORES):
        lo = c * SHARD
        hi = min(lo + SHARD, N)
        n = hi - lo
        sqc = np.zeros(NTROWS, np.float32)
        dvc = np.zeros(NTROWS, np.float32)
        sqc[:n] = np.sqrt(deg[lo:hi]).astype(np.float32)
        dvc[:n] = dinv[lo:hi].astype(np.float32)
        for t in range(NT):
            sq_pack[c, 32 * (t % 3), (t // 3) * P:(t // 3 + 1) * P] = \
                sqc[t * P:(t + 1) * P]
        m = dvc.reshape(NT, P).T
        dinv_cols[c] = m
        scale_cols[c, :, 0 * NT:1 * NT] = m
        scale_cols[c, :, 1 * NT:2 * NT] = m * m
        scale_cols[c, :, 2 * NT:3 * NT] = m * m
    return dict(
        batches=batches, first_q=first_q, last_q=last_q, gb_max=gb_max,
        tot16=TOT // 16, totg=TOT // P,
        idx_streams=idx_streams, dl_streams=dl_streams,
        sq_pack=sq_pack, scale_cols=scale_cols, dinv_cols=dinv_cols,
    )


# ---------------------------------------------------------------------------
# Kernel builder
# ---------------------------------------------------------------------------

def build_kernel(meta, cfg):
    import concourse.bacc as bacc
    import concourse.bass as bass
    import concourse.mybir as mybir
    import concourse.tile as tile

    f32 = mybir.dt.float32
    i16 = mybir.dt.int16
    NT, NTROWS = cfg["NT"], cfg["NTROWS"]
    QROWS, NQ, PADN, CROWS = cfg["QROWS"], cfg["NQ"], cfg["PADN"], cfg["CROWS"]
    NC3 = -(-NT // 3)
    GBMAX = meta["gb_max"]
    batches = meta["batches"]
    first_q, last_q = meta["first_q"], meta["last_q"]

    nc = bacc.Bacc("TRN2", target_bir_lowering=False, debug=False,
                   num_devices=NCORES, num_swdge_queues=4)

    # I/O
    feat_t = nc.dram_tensor("feat_t", [P, NTROWS], f32, kind="ExternalInput")
    idx_in = nc.dram_tensor("idx_in", [16, meta["tot16"]], i16, kind="ExternalInput")
    dl_in = nc.dram_tensor("dl_in", [P, meta["totg"]], i16, kind="ExternalInput")
    sq_in = nc.dram_tensor("sq_in", [65, NC3 * P], f32, kind="ExternalInput")
    sc_in = nc.dram_tensor("sc_in", [P, L * NT], f32, kind="ExternalInput")
    dv_in = nc.dram_tensor("dv_in", [P, NT], f32, kind="ExternalInput")
    wc_in = nc.dram_tensor("wc_in", [L, P, P], f32, kind="ExternalInput")
    wl_in = nc.dram_tensor("wl_in", [P, L * C], f32, kind="ExternalInput")
    bc_in = nc.dram_tensor("bc_in", [65, L * P], f32, kind="ExternalInput")
    bl_in = nc.dram_tensor("bl_in", [P, C], f32, kind="ExternalInput")
    out_t = nc.dram_tensor("out_t", [P, NT * C], f32, kind="ExternalOutput")

    # internal DRAM for the collective tables
    cc_in = [nc.dram_tensor(f"ccin{l}", [PADN, D], f32) for l in range(L)]
    cc_out = [[nc.dram_tensor(f"ccout{l}_{q}", [CROWS, D], f32,
                              addr_space="Shared") for q in range(NQ)]
              for l in range(L)]

    rg = [list(range(NCORES))]
    AF = mybir.ActivationFunctionType
    OP = mybir.AluOpType

    with tile.TileContext(nc) as tc:
        with (
            tc.tile_pool(name="const", bufs=1) as pc,
            tc.tile_pool(name="gath", bufs=2) as pg,
            tc.tile_pool(name="oh", bufs=2) as po,
            tc.tile_pool(name="hstage", bufs=2) as ph,
            tc.tile_pool(name="pagg", bufs=6, space="PSUM") as pa,
            tc.tile_pool(name="pmisc", bufs=2, space="PSUM") as pm,
        ):
            # ---- constants ----
            # dma_gather index data: wrapped into 16 partitions and replicated
            # across the 8 gpsimd cores' partition groups (each Q7 core reads
            # its own [16k, 16k+16) window)
            idx_res = pc.tile([P, meta["tot16"]], i16)
            for k in range(8):
                nc.sync.dma_start(out=idx_res[16 * k:16 * (k + 1), :],
                                  in_=idx_in[:, :])
            dl_res = pc.tile([P, meta["totg"]], i16)
            nc.sync.dma_start(out=dl_res[:, :], in_=dl_in[:, :])
            sq_t = pc.tile([65, NC3 * P], f32)
            nc.sync.dma_start(out=sq_t[:, :], in_=sq_in[:, :])
            sc_t = pc.tile([P, L * NT], f32)
            nc.sync.dma_start(out=sc_t[:, :], in_=sc_in[:, :])
            dv_t = pc.tile([P, NT], f32)
            nc.sync.dma_start(out=dv_t[:, :], in_=dv_in[:, :])
            wc_t = pc.tile([P, L * P], f32)
            for l in range(L):
                nc.sync.dma_start(out=wc_t[:, l * P:(l + 1) * P], in_=wc_in[l])
            wl_t = pc.tile([P, L * C], f32)
            nc.sync.dma_start(out=wl_t[:, :], in_=wl_in[:, :])
            bc_t = pc.tile([65, L * P], f32)
            nc.sync.dma_start(out=bc_t[:, :], in_=bc_in[:, :])
            bl_t = pc.tile([P, C], f32)
            nc.sync.dma_start(out=bl_t[:, :], in_=bl_in[:, :])
            iota_t = pc.tile([P, P], i16)
            nc.gpsimd.iota(iota_t[:, :], pattern=[[1, P]], base=0,
                           channel_multiplier=0)

            xt = pc.tile([P, NTROWS], f32)           # x~ (feature-major)
            logits = pc.tile([P, NT * C], f32)
            nc.vector.memset(logits[:, :], 0.0)

            for l in range(L):
                # ---- transform: H' = scale * (x @ Wc[l]), quarter-pipelined
                if l == 0:
                    nc.sync.dma_start(out=xt[:, :], in_=feat_t[:, :])
                wc_l = wc_t[:, l * P:(l + 1) * P]
                quads = list(range(0, NT, 4))
                # quad index after which each quarter is complete
                ag_after = {}
                for qq in range(NQ):
                    lastrow = min((qq + 1) * QROWS, NTROWS)
                    lastt = (lastrow - 1) // P
                    ag_after.setdefault(min(lastt // 4, len(quads) - 1), []).append(qq)
                for qi, t0 in enumerate(quads):
                    ts = list(range(t0, min(t0 + 4, NT)))
                    nts = len(ts)
                    hp = pm.tile([P, 512], f32, tag="misc")
                    for i, t in enumerate(ts):
                        nc.tensor.matmul(
                            hp[:, i * P:(i + 1) * P],
                            lhsT=xt[:, t * P:(t + 1) * P],
                            rhs=wc_l, start=True, stop=True)
                    hs = ph.tile([P, 512], f32)
                    for i, t in enumerate(ts):
                        nc.scalar.activation(
                            out=hs[:, i * P:(i + 1) * P],
                            in_=hp[:, i * P:(i + 1) * P],
                            func=AF.Copy,
                            scale=sc_t[:, l * NT + t:l * NT + t + 1])
                    dst_ap = cc_in[l][t0 * P:(t0 + nts) * P, :].rearrange(
                        "(a p) f -> p a f", p=P)
                    src_ap = hs[:, :nts * P].rearrange("p (a f) -> p a f", f=P)
                    nc.sync.dma_start(out=dst_ap, in_=src_ap)
                    for qq in ag_after.get(qi, []):
                        nc.gpsimd.collective_compute(
                            "AllGather", OP.bypass, replica_groups=rg,
                            ins=[cc_in[l][qq * QROWS:(qq + 1) * QROWS, :]],
                            outs=[cc_out[l][qq][:, :]])

                # ---- aggregation ----
                for bi, B in enumerate(batches):
                    qq, ni = B["q"], B["ni"]
                    gb = ni // P
                    gt = pg.tile([P, GBMAX * P], f32, tag="gath")
                    # single_packet coalesces each engine's descriptors into
                    # one SDMA packet; the HW packet limit is 64 descriptors,
                    # so only use it when ni/16 + 1 <= 64
                    nc.gpsimd.dma_gather(
                        out_ap=gt[:, :gb * P].rearrange("p (g f) -> p g f", f=P),
                        in_ap=cc_out[l][qq][:, :],
                        idxs_ap=idx_res[:, B["icol"]:B["icol"] + ni // 16],
                        num_idxs=ni, num_idxs_reg=ni, elem_size=P,
                        single_packet=(ni <= 1008),
                        queue_num=bi % 4)
                    oh = po.tile([P, GBMAX * P], f32, tag="oh")
                    nc.vector.tensor_tensor(
                        out=oh[:, :gb * P].rearrange("p (g d) -> p g d", d=P),
                        in0=dl_res[:, B["gcol"]:B["gcol"] + gb].to_broadcast(
                            [P, gb, P]),
                        in1=iota_t[:, None, :].to_broadcast([P, gb, P]),
                        op=OP.is_equal)
                    goff = 0
                    for (t, gcnt) in B["tiles"]:
                        ps = pa.tile([P, P], f32, tag="agg")
                        is_last = qq == last_q[t]
                        for g in range(gcnt):
                            sl = slice((goff + g) * P, (goff + g + 1) * P)
                            nc.tensor.matmul(
                                ps[:, :], lhsT=gt[:, sl], rhs=oh[:, sl],
                                start=(g == 0),
                                stop=(g == gcnt - 1 and not is_last))
                        xs = xt[:, t * P:(t + 1) * P]
                        if is_last:
                            bp = 32 * (t % 3)
                            nc.tensor.matmul(
                                ps[:, :],
                                lhsT=bc_t[bp:bp + 1, l * P:(l + 1) * P],
                                rhs=sq_t[bp:bp + 1,
                                         (t // 3) * P:(t // 3 + 1) * P],
                                start=False, stop=True)
                        if qq == first_q[t]:
                            nc.vector.tensor_copy(out=xs, in_=ps[:, :])
                        else:
                            nc.vector.tensor_add(out=xs, in0=xs, in1=ps[:, :])
                        if is_last:
                            nc.scalar.activation(out=xs, in_=xs, func=AF.Relu)
                            lp = pm.tile([P, 512], f32, tag="misc")
                            nc.tensor.matmul(
                                lp[:, :C], lhsT=xs,
                                rhs=wl_t[:, l * C:(l + 1) * C],
                                start=True, stop=True)
                            nc.vector.tensor_add(
                                out=logits[:, t * C:(t + 1) * C],
                                in0=logits[:, t * C:(t + 1) * C],
                                in1=lp[:, :C])
                        goff += gcnt

            # ---- final: logits = dinv*logits + bl; log_softmax ----
            work = pc.tile([P, NT * C], f32)
            ework = pc.tile([P, NT * C], f32)
            red = pc.tile([P, NT], f32)
            red2 = pc.tile([P, NT], f32)
            w3 = work[:, :].rearrange("p (t c) -> p t c", c=C)
            e3 = ework[:, :].rearrange("p (t c) -> p t c", c=C)
            l3 = logits[:, :].rearrange("p (t c) -> p t c", c=C)
            nc.vector.tensor_tensor(out=w3, in0=l3,
                                    in1=dv_t[:, :].to_broadcast([P, NT, C]),
                                    op=OP.mult)
            nc.vector.tensor_tensor(out=w3, in0=w3,
                                    in1=bl_t[:, None, :].to_broadcast([P, NT, C]),
                                    op=OP.add)
            nc.vector.tensor_reduce(out=red[:, :], in_=w3,
                                    axis=mybir.AxisListType.X, op=OP.max,
                                    negate=True)
            nc.vector.tensor_tensor(out=w3, in0=w3,
                                    in1=red[:, :].to_broadcast([P, NT, C]),
                                    op=OP.add)
            nc.scalar.activation(out=e3, in_=w3, func=AF.Exp)
            nc.vector.tensor_reduce(out=red2[:, :], in_=e3,
                                    axis=mybir.AxisListType.X, op=OP.add)
            nc.scalar.activation(out=red2[:, :], in_=red2[:, :], func=AF.Ln)
            nc.vector.tensor_tensor(out=w3, in0=w3,
                                    in1=red2[:, :].to_broadcast([P, NT, C]),
                                    op=OP.subtract)
            nc.sync.dma_start(out=out_t[:, :], in_=work[:, :])

    nc.compile()
    return nc


# ---------------------------------------------------------------------------
# Host-side input/output marshalling
# ---------------------------------------------------------------------------

def make_in_maps(feat, edge_index, Wc, bc, Wl, bl, meta, cfg):
    N, SHARD, NT, NTROWS = cfg["N"], cfg["SHARD"], cfg["NT"], cfg["NTROWS"]
    feat = np.ascontiguousarray(np.asarray(feat, np.float32))
    Wc = np.asarray(Wc, np.float32)
    bc = np.asarray(bc, np.float32)
    Wl = np.asarray(Wl, np.float32).reshape(L, P, C)
    bl = np.asarray(bl, np.float32)

    wl_pack = np.ascontiguousarray(np.concatenate([Wl[l] for l in range(L)], axis=1))
    bl_rep = np.ascontiguousarray(np.broadcast_to(bl[None, :], (P, C)))
    bc_pack = np.zeros((65, L * P), np.float32)
    for bp in (0, 32, 64):
        bc_pack[bp] = bc.reshape(-1)

    in_maps = []
    for c in range(NCORES):
        lo = c * SHARD
        hi = min(lo + SHARD, N)
        f = np.zeros((NTROWS, D), np.float32)
        f[:hi - lo] = feat[lo:hi]
        in_maps.append({
            "feat_t": np.ascontiguousarray(f.T),
            "idx_in": np.ascontiguousarray(meta["idx_streams"][c]),
            "dl_in": np.ascontiguousarray(meta["dl_streams"][c]),
            "sq_in": np.ascontiguousarray(meta["sq_pack"][c]),
            "sc_in": np.ascontiguousarray(meta["scale_cols"][c]),
            "dv_in": np.ascontiguousarray(meta["dinv_cols"][c]),
            "wc_in": np.ascontiguousarray(Wc),
            "wl_in": wl_pack,
            "bc_in": bc_pack,
            "bl_in": bl_rep,
        })
    return in_maps


def assemble_output(results, cfg):
    N, SHARD, NT = cfg["N"], cfg["SHARD"], cfg["NT"]
    out = np.zeros((N, C), np.float32)
    for c, res in enumerate(results):
        o = res["out_t"].reshape(P, NT, C).transpose(1, 0, 2).reshape(NT * P, C)
        lo = c * SHARD
        hi = min(lo + SHARD, N)
        out[lo:hi] = o[:hi - lo]
    return out


_CACHE = {}


def kernel(feat, edge_index, Wc, bc, Wl, bl):
    from concourse.bass_utils import run_bass_kernel_spmd

    cfg = FULL_CFG
    meta = preprocess(edge_index, cfg)
    nc = build_kernel(meta, cfg)
    in_maps = make_in_maps(feat, edge_index, Wc, bc, Wl, bl, meta, cfg)
    res = run_bass_kernel_spmd(nc, in_maps, core_ids=list(range(NCORES)),
                               trace=bool(int(os.environ.get("GCN_TRACE", "0"))))
    return assemble_output(res.results, cfg)
